# revision 9
# baseline (speedup 1.0000x reference)
"""Causal self-attention (B=1, T=4096, C=1024, H=16, D=64) on 8 NeuronCores.

Sharding: tensor-parallel over heads. Core i handles heads (2i, 2i+1):
it computes q/k/v projections for its 128 qkv columns, attention for its
2 heads, and a partial output projection (rank-128 slice of the
contraction). The host sums the 8 partial outputs and adds b_proj.

v2 layout/scheduling notes:
  - scores for a 128-k block are emitted as FOUR K=64/M=64 matmuls whose
    tile positions (auto-derived from base partitions) occupy the four
    PE-array quadrants, so both heads' scores for the block stream
    concurrently in ~512 cycles instead of ~1024.
  - the attention @ v result is normalized BEFORE the output projection:
    the softmax denominators (ones-column of av) are inverted with the
    fast DVE reciprocal and broadcast across partitions with a K=2
    matmul against a [2,128] selector, letting the projection contract
    both heads in one K=128 matmul group.
  - ACT does exp only; every PSUM->SBUF move runs on DVE.
  - qkv projection tiles are interleaved into the previous attention
    q-tile's instruction stream, and xT input DMAs are j-major so the
    first projection starts ~5us in.
"""

import sys

if "/opt/trn_rl_repo" not in sys.path:
    sys.path.insert(0, "/opt/trn_rl_repo")

import numpy as np
import ml_dtypes

T = 4096
C = 1024
H = 16
D = 64
NCORES = 8
HPC = H // NCORES  # heads per core = 2
QT = 512  # q-tile width
NQT = T // QT  # 8
KB = 128  # k-block
NKB = T // KB  # 32
BF16 = ml_dtypes.bfloat16
OUT_BF16 = True  # partial outputs in bf16 (summed in f32 on host)

_COMPILED = {}


def _build_nc(with_bias=True):
    import concourse.tile as tile
    from concourse import bacc, mybir

    F32 = mybir.dt.float32
    BF = mybir.dt.bfloat16
    ODT = BF if OUT_BF16 else F32
    Exp = mybir.ActivationFunctionType.Exp

    nc = bacc.Bacc("TRN2", target_bir_lowering=False, debug=False,
                   num_devices=NCORES)

    def din(name, shape, dt=BF):
        if dt is None:
            dt = F32
        return nc.dram_tensor(name, shape, dt, kind="ExternalInput").ap()

    xT = din("xT", [C, T])                 # x transposed, bf16
    wq = din("wq", [128, C])               # packed: [c%128, (c//128)*128 + m]
    wk = din("wk", [128, C])
    wv = din("wv", [128, C])
    wp = din("wp", [128, C])               # w_proj rows for this core's dims
    bq = din("bq", [1, 128])
    bk = din("bk", [1, 128])
    bv = din("bv", [1, 128])
    ones = din("ones", [1, QT])
    ident = din("ident", [128, 64])        # I64 stacked twice
    onesf = din("onesf", [1, 128], dt=None)  # f32 ones (broadcast lhsT)
    masks = [din(f"mask{d}", [128, 2 * QT]) for d in range(4)]  # [m_d | m_d]
    out = nc.dram_tensor("out", [T, C], ODT, kind="ExternalOutput").ap()

    with tile.TileContext(nc) as tc:
        with (
            tc.tile_pool(name="const", bufs=1) as cpool,
            tc.tile_pool(name="qkv", bufs=1) as qkvpool,
            tc.tile_pool(name="exp", bufs=6) as epool,
            tc.tile_pool(name="small", bufs=2) as spool,
            tc.tile_pool(name="ostage", bufs=3) as opool,
            tc.tile_pool(name="ps_sc", bufs=2, space="PSUM") as ps_sc,
            tc.tile_pool(name="ps_sm", bufs=2, space="PSUM") as ps_sm,
            tc.tile_pool(name="ps_avA", bufs=1, space="PSUM") as ps_avA,
            tc.tile_pool(name="ps_avB", bufs=1, space="PSUM") as ps_avB,
        ):
            # ---- resident inputs. Weights/masks first on the sync ring
            # (they gate qkv tile 0); xT follows j-major split across the
            # scalar+vector rings so qkv tile j never waits on tile >j's
            # bytes. Output DMAs go on the sync ring, which is idle once
            # the small constants are in. ----
            w_sb = {}
            for nm, t in (("wq", wq), ("wk", wk), ("wv", wv), ("wp", wp)):
                w_sb[nm] = cpool.tile([128, C], BF, tag=nm, name=nm)
                nc.sync.dma_start(w_sb[nm][:], t[:])
            b_sb = {}
            for nm, t in (("bq", bq), ("bk", bk), ("bv", bv)):
                b_sb[nm] = cpool.tile([1, 128], BF, tag=nm, name=nm)
                nc.sync.dma_start(b_sb[nm][:], t[:])
            ones_sb = cpool.tile([1, QT], BF, tag="ones")
            nc.sync.dma_start(ones_sb[:], ones[:])
            ident_sb = cpool.tile([128, 64], BF, tag="ident")
            nc.sync.dma_start(ident_sb[:], ident[:])
            onesf_sb = cpool.tile([1, 128], F32, tag="onesf")
            nc.sync.dma_start(onesf_sb[:], onesf[:])
            m_sb = []
            for d in range(4):
                mt = cpool.tile([128, 2 * QT], BF, tag=f"mask{d}",
                                name=f"mask{d}")
                nc.sync.dma_start(mt[:], masks[d][:])
                m_sb.append(mt)
            xT_sb = cpool.tile([128, 8, T], BF, tag="xT")
            xTv = xT.rearrange("(k p) t -> p k t", p=128)
            for j in range(NQT):
                nc.sync.dma_start(
                    xT_sb[:, :, j * QT:(j + 1) * QT],
                    xTv[:, :, j * QT:(j + 1) * QT])

            qT_sb = qkvpool.tile([128, T], BF, tag="qT")
            kT_sb = qkvpool.tile([128, T], BF, tag="kT")
            vT_sb = qkvpool.tile([128, T], BF, tag="vT")
            vstore = []
            for h in range(2):
                vs = qkvpool.tile([128, NKB, 65], BF, tag=f"vst{h}",
                                  name=f"vst{h}")
                nc.gpsimd.memset(vs[:, :, 64], 1.0)
                vstore.append(vs)

            # ---- qkv work units (one j-tile = 3 projections + 4 v'
            # transpose blocks), emitted piecemeal into the previous
            # attention tile's stream ----
            def qkv_unit(wt, bias, dst, j):
                ps = ps_sm.tile([128, QT], F32, tag="ps", name="psqkv")
                for c0 in range(8):
                    nc.tensor.matmul(
                        ps[:],
                        lhsT=w_sb[wt][:, c0 * 128:(c0 + 1) * 128],
                        rhs=xT_sb[:, c0, j * QT:(j + 1) * QT],
                        start=(c0 == 0),
                        stop=(not with_bias and c0 == 7))
                if with_bias:
                    nc.tensor.matmul(ps[:], lhsT=b_sb[bias][:],
                                     rhs=ones_sb[:], start=False, stop=True)
                nc.vector.tensor_copy(dst[:, j * QT:(j + 1) * QT], ps[:])

            def vprime_unit(blk):
                for h in range(2):
                    pt = ps_sm.tile([128, 64], BF, tag="ps", name="pt")
                    nc.tensor.transpose(
                        pt[:, 0:64],
                        vT_sb[h * 64:(h + 1) * 64, blk * 128:(blk + 1) * 128],
                        ident_sb[h * 64:(h + 1) * 64, :])
                    nc.vector.tensor_copy(vstore[h][:, blk, 0:64],
                                          pt[:, 0:64])

            def qkv_tile_units(j):
                units = [
                    lambda j=j: qkv_unit("wv", "bv", vT_sb, j),
                    lambda j=j: qkv_unit("wk", "bk", kT_sb, j),
                    lambda j=j: qkv_unit("wq", "bq", qT_sb, j),
                ]
                for c in range(4):
                    units.append(lambda blk=4 * j + c: vprime_unit(blk))
                return units

            # ---- attention pieces ----
            def emit_scores(i, b):
                """scores block b (both heads, 4 PE quadrants) -> exp/mask."""
                ps = ps_sc.tile([128, 2 * QT], F32, tag="sc", name="sc")
                for h in range(2):
                    hs = slice(h * 64, (h + 1) * 64)
                    for half in range(2):
                        k0 = b * 128 + half * 64
                        nc.tensor.matmul(
                            ps[half * 64:half * 64 + 64,
                               h * QT:(h + 1) * QT],
                            lhsT=kT_sb[hs, k0:k0 + 64],
                            rhs=qT_sb[hs, i * QT:(i + 1) * QT],
                            start=True, stop=True)
                et = epool.tile([128, 2 * QT], BF, tag="exp", name="et")
                d = b - 4 * i  # diagonal-block offset /128
                if d in (2, 3):
                    off = 128 * d
                    etv = et[:].rearrange("p (h q) -> p h q", h=2)
                    psv = ps[:].rearrange("p (h q) -> p h q", h=2)
                    mv = m_sb[d][:].rearrange("p (h q) -> p h q", h=2)
                    nc.gpsimd.memset(etv[:, :, 0:off], 0.0)
                    nc.scalar.activation(etv[:, :, off:QT], psv[:, :, off:QT],
                                         Exp, scale=0.125)
                    nc.vector.tensor_mul(etv[:, :, off:QT], etv[:, :, off:QT],
                                         mv[:, :, off:QT])
                else:
                    nc.scalar.activation(et[:], ps[:], Exp, scale=0.125)
                    if d in (0, 1):
                        nc.vector.tensor_mul(et[:], et[:], m_sb[d][:])
                return et

            def emit_av(i, b, et, avA, avB, nblk):
                for h, av in ((0, avA), (1, avB)):
                    nc.tensor.matmul(
                        av[0:65, :],
                        lhsT=vstore[h][:, b, :],
                        rhs=et[:, h * QT:(h + 1) * QT],
                        start=(b == 0), stop=(b == nblk - 1))

            def tail_sums(i, avA, avB):
                """denominators -> 1/x -> broadcast -> normalized u."""
                s2 = spool.tile([1, 2 * QT], F32, tag="s2", name="s2")
                nc.vector.tensor_copy(s2[0:1, 0:QT], avA[64:65, :])
                nc.vector.tensor_copy(s2[0:1, QT:2 * QT], avB[64:65, :])
                r2 = spool.tile([1, 2 * QT], F32, tag="r2", name="r2")
                nc.vector.reciprocal_approx_fast(r2[:], s2[:])
                rp = ps_sm.tile([128, QT], F32, tag="ps", name="rp")
                for h in range(2):
                    nc.tensor.matmul(rp[h * 64:(h + 1) * 64, :],
                                     lhsT=onesf_sb[0:1, 0:64],
                                     rhs=r2[0:1, h * QT:(h + 1) * QT],
                                     start=True, stop=True)
                rr = spool.tile([128, QT], F32, tag="rr", name="rr")
                nc.vector.tensor_copy(rr[:], rp[:])
                u = spool.tile([128, QT], BF, tag="u", name="u")
                nc.vector.tensor_mul(u[0:64, :], avA[0:64, :], rr[0:64, :])
                nc.vector.tensor_mul(u[64:128, :], avB[0:64, :],
                                     rr[64:128, :])
                return u

            def tail_proj(i, u):
                for cchunk in range(4):
                    qs = slice(cchunk * 128, (cchunk + 1) * 128)
                    ost = opool.tile([128, C], ODT, tag="ost", name="ost")
                    for chalf in range(2):
                        cs = slice(chalf * QT, (chalf + 1) * QT)
                        pp = ps_sm.tile([128, QT], F32, tag="ps", name="pp")
                        nc.tensor.matmul(pp[:], lhsT=u[:, qs],
                                         rhs=w_sb["wp"][:, cs],
                                         start=True, stop=True)
                        nc.vector.tensor_copy(ost[:, cs], pp[:])
                    row = i * QT + cchunk * 128
                    nc.gpsimd.dma_start(out[row:row + 128, :], ost[:])

            # ---- main loop: qkv(0) up front, then per q-tile attention
            # with qkv(i+1) injected into the spare PE slots ----
            for fn in qkv_tile_units(0):
                fn()

            pend_scale = None  # (i, avA, avB) awaiting tail_sums/proj
            pend_proj = None
            for i in range(NQT):
                avA = ps_avA.tile([128, QT], F32, tag="avA", name="avA")
                avB = ps_avB.tile([128, QT], F32, tag="avB", name="avB")
                nblk = 4 * (i + 1)
                inject = qkv_tile_units(i + 1) if i + 1 < NQT else []
                pend_av = None  # (b, et)
                for b in range(nblk):
                    et = emit_scores(i, b)
                    if b == 0 and pend_scale is not None:
                        pi, pA, pB = pend_scale
                        with tc.high_priority():
                            pu = tail_sums(pi, pA, pB)
                        pend_scale = None
                        pend_proj = (pi, pu)
                    if pend_av is not None:
                        emit_av(i, pend_av[0], pend_av[1], avA, avB, nblk)
                    if b == 1 and pend_proj is not None:
                        tail_proj(*pend_proj)
                        pend_proj = None
                    if b >= 2 and inject:
                        inject.pop(0)()
                    pend_av = (b, et)
                emit_av(i, pend_av[0], pend_av[1], avA, avB, nblk)
                while inject:
                    inject.pop(0)()
                pend_scale = (i, avA, avB)
                pend_proj = None
            # final tail
            pi, pA, pB = pend_scale
            pu = tail_sums(pi, pA, pB)
            tail_proj(pi, pu)

    nc.compile()
    return nc


def _causal_mask(d):
    kp = np.arange(128)[:, None]
    qf = np.arange(QT)[None, :]
    return ((kp + d) <= qf).astype(BF16)


def _prep_inputs(x, w_qkv, b_qkv, w_proj):
    """Build the 8 per-core input maps (host-side shard + pack)."""
    xT = np.ascontiguousarray(x.reshape(T, C).T).astype(BF16)
    masks = {}
    for d in range(4):
        m = _causal_mask(128 * d)
        masks[f"mask{d}"] = np.concatenate([m, m], axis=1)
    ident = np.zeros((128, 64), dtype=BF16)
    ident[np.arange(128), np.arange(128) % 64] = 1
    ones = np.ones((1, QT), dtype=BF16)
    onesf = np.ones((1, 128), dtype=np.float32)

    def pack_w(wcols):  # [C, 128] -> [128, C] chunk-packed for SBUF
        return np.ascontiguousarray(
            wcols.reshape(8, 128, 128).transpose(1, 0, 2).reshape(128, C)
        ).astype(BF16)

    in_maps = []
    for core in range(NCORES):
        h0 = core * HPC
        cols = slice(h0 * D, (h0 + HPC) * D)  # 128 cols for this core
        m = {
            "xT": xT,
            "wq": pack_w(w_qkv[:, :C][:, cols]),
            "wk": pack_w(w_qkv[:, C:2 * C][:, cols]),
            "wv": pack_w(w_qkv[:, 2 * C:][:, cols]),
            "wp": np.ascontiguousarray(w_proj[cols, :]).astype(BF16),
            "bq": b_qkv[:C][cols].reshape(1, 128).astype(BF16),
            "bk": b_qkv[C:2 * C][cols].reshape(1, 128).astype(BF16),
            "bv": b_qkv[2 * C:][cols].reshape(1, 128).astype(BF16),
            "ones": ones,
            "ident": ident,
            "onesf": onesf,
        }
        m.update(masks)
        in_maps.append(m)
    return in_maps


def _get_compiled(with_bias=True):
    if with_bias not in _COMPILED:
        _COMPILED[with_bias] = _build_nc(with_bias=with_bias)
    return _COMPILED[with_bias]


def run_on_device(in_maps, with_bias=True, **kwargs):
    from concourse.bass_utils import run_bass_kernel_spmd

    nc = _get_compiled(with_bias)
    return run_bass_kernel_spmd(nc, in_maps, core_ids=list(range(NCORES)),
                                **kwargs)


def kernel(x, w_qkv, b_qkv, w_proj, b_proj, **run_kwargs):
    x = np.asarray(x, dtype=np.float32)
    w_qkv = np.asarray(w_qkv, dtype=np.float32)
    b_qkv = np.asarray(b_qkv, dtype=np.float32)
    w_proj = np.asarray(w_proj, dtype=np.float32)
    b_proj = np.asarray(b_proj, dtype=np.float32)

    in_maps = _prep_inputs(x, w_qkv, b_qkv, w_proj)
    with_bias = bool(np.any(b_qkv))
    res = run_on_device(in_maps, with_bias=with_bias, **run_kwargs)
    acc = np.zeros((T, C), dtype=np.float32)
    for core in range(NCORES):
        acc += np.asarray(res.results[core]["out"], dtype=np.float32)
    acc += b_proj[None, :]
    out = acc.reshape(1, T, C)
    kernel.last_results = res
    return out
